# revision 1
# baseline (speedup 1.0000x reference)
"""HardNegativeMiningContrastiveLoss on 8 trn2 NeuronCores (Bass/Tile).

Strategy:
  - Host: sort rows of both feature matrices by match_id. Since rows and
    columns share the same match_ids, the match matrix becomes block
    diagonal: all matches for (sorted) row i lie within +-(m*-1) columns
    of i, where m* = max id multiplicity. Each core owns a 512-row block
    of anchors for BOTH directions (v2t / t2v). The rhs (all 4096
    columns, transposed for matmul) is rotated per-core so the match
    band of local row-tile r sits at columns [128r, 128r+W) -- a uniform
    offset, which keeps the program SPMD.
  - Device: sim row-block via PE matmul (fp32), exp row-sums on ACT with
    fused accumulation, semi-hard range sums via fused
    scalar_tensor_tensor on DVE/GPSIMD, and all match-masked terms
    (mean_pos, corrections, the -log(p) keep terms) computed only on the
    narrow diagonal band.
  - Host: valid-row mask, final scalar reduction.
"""

import numpy as np

import concourse.bass as bass
import concourse.bacc as bacc
import concourse.tile as tile
from concourse import mybir
from concourse.bass_utils import run_bass_kernel_spmd
from contextlib import ExitStack

N_CORES = 8
B = 4096
D = 512
BLK = B // N_CORES  # 512 anchors per core
TEMPERATURE = 0.07
SEMI_HARD_MARGIN = 0.2
EPS = 1e-12

F32 = mybir.dt.float32
AX = mybir.AxisListType.X
ALU = mybir.AluOpType
ACTF = mybir.ActivationFunctionType

_CACHE = {}


def _build(shift: int, w: int, repeat: int = 1, loads_in_loop: bool = True):
    """Build + compile the SPMD program. w = band width, shift = column
    rotation applied on host (band of row-tile r = cols [128r, 128r+w)).
    repeat>1 replays the full load+compute pipeline (measurement only)."""
    nc = bacc.Bacc("TRN2", target_bir_lowering=False, debug=False,
                   num_devices=N_CORES)

    # Inputs (per-core values differ; shapes identical -> SPMD).
    rhs_t = nc.dram_tensor("rhs_t", [D, B], F32, kind="ExternalInput")
    rhs_v = nc.dram_tensor("rhs_v", [D, B], F32, kind="ExternalInput")
    ids_bcd = nc.dram_tensor("ids_bcd", [128, BLK + w], F32,
                             kind="ExternalInput")
    ids_rows = nc.dram_tensor("ids_rows", [128, 4], F32, kind="ExternalInput")
    inv_cnt = nc.dram_tensor("inv_cnt", [128, 4], F32, kind="ExternalInput")
    ks_out = nc.dram_tensor("ks_out", [128, 8], F32, kind="ExternalOutput")

    invT = float(1.0 / TEMPERATURE)
    NKC = D // 128   # 4 contraction chunks
    NCT = B // 512   # 8 column tiles
    NRT = BLK // 128  # 4 row tiles

    with tile.TileContext(nc) as tc, ExitStack() as ctx:
        rhs_pool = ctx.enter_context(tc.tile_pool(name="rhs", bufs=8))
        e_pool = ctx.enter_context(tc.tile_pool(name="erow", bufs=2))
        psum = ctx.enter_context(
            tc.tile_pool(name="psum", bufs=8, space=bass.MemorySpace.PSUM))
        scratch = ctx.enter_context(tc.tile_pool(name="scr", bufs=2))
        band_pool = ctx.enter_context(tc.tile_pool(name="band", bufs=3))
        small = ctx.enter_context(tc.tile_pool(name="small", bufs=6))
        const_pool = ctx.enter_context(tc.tile_pool(name="const", bufs=1))

        # Column ids broadcast across partitions (host-replicated).
        ids_bc = const_pool.tile([128, BLK + w], F32, tag="idsbc")
        nc.sync.dma_start(ids_bc[:], ids_bcd[:])

        # Per-row-tile ids / inv_cnt as [128,1] columns.
        ids_r = const_pool.tile([128, NRT], F32, tag="idsr")
        nc.sync.dma_start(ids_r[:], ids_rows[:])
        icnt_r = const_pool.tile([128, NRT], F32, tag="icntr")
        nc.sync.dma_start(icnt_r[:], inv_cnt[:])

        ks_cols = const_pool.tile([128, 2 * NRT], F32, tag="kscols")

        def load_rhs():
            rt_tiles, rv_tiles = [], []
            for k in range(NKC):
                t = rhs_pool.tile([128, B], F32, tag="rhs")
                nc.sync.dma_start(t[:], rhs_t[bass.ts(k, 128), :])
                rt_tiles.append(t)
            for k in range(NKC):
                t = rhs_pool.tile([128, B], F32, tag="rhs")
                nc.sync.dma_start(t[:], rhs_v[bass.ts(k, 128), :])
                rv_tiles.append(t)
            return rt_tiles, rv_tiles

        if not loads_in_loop:
            rt_tiles, rv_tiles = load_rhs()
        for rep in range(repeat):
          if loads_in_loop:
              rt_tiles, rv_tiles = load_rhs()

          for d in range(2):
              rh = rt_tiles if d == 0 else rv_tiles
              lsrc = rv_tiles if d == 0 else rt_tiles
              lh = [t[:, shift:shift + BLK] for t in lsrc]

              for r in range(NRT):
                  erow = e_pool.tile([128, B], F32, tag="erow")
                  sband = band_pool.tile([128, w], F32, tag="sband")
                  sl_e = small.tile([128, NCT], F32, tag="sl_e")
                  sl_1 = small.tile([128, NCT], F32, tag="sl_1")
                  sl_2 = small.tile([128, NCT], F32, tag="sl_2")

                  # mean_pos mask for the diagonal band.
                  bnd = slice(128 * r, 128 * r + w)
                  m_band = band_pool.tile([128, w], F32, tag="m")
                  nc.vector.tensor_scalar(
                      m_band[:], ids_bc[:, bnd], ids_r[:, r:r + 1], None,
                      op0=ALU.is_equal)
                  mp = small.tile([128, 1], F32, tag="mp")
                  mp2 = small.tile([128, 1], F32, tag="mp2")
                  pos_s = small.tile([128, 1], F32, tag="poss")

                  bsplit = min(128 * r + w, 512) - 128 * r  # band cols in c=0
                  nband = 1 if bsplit == w else 2

                  def do_matmul(c):
                      p = psum.tile([128, 512], F32, tag="p")
                      for k in range(NKC):
                          nc.tensor.matmul(
                              p[:], lh[k][:, bass.ts(r, 128)],
                              rh[k][:, bass.ts(c, 512)],
                              start=(k == 0), stop=(k == NKC - 1))
                      return p

                  def consume(c, p):
                      csl = bass.ts(c, 512)
                      nc.scalar.activation(
                          erow[:, csl], p[:], ACTF.Exp, scale=invT,
                          accum_out=sl_e[:, c:c + 1])
                      s1 = scratch.tile([128, 512], F32, tag="s1")
                      nc.vector.scalar_tensor_tensor(
                          out=s1[:], in0=p[:], scalar=mp[:],
                          in1=erow[:, csl], op0=ALU.is_lt, op1=ALU.mult,
                          accum_out=sl_1[:, c:c + 1])
                      s2 = scratch.tile([128, 512], F32, tag="s2")
                      nc.vector.scalar_tensor_tensor(
                          out=s2[:], in0=p[:], scalar=mp2[:],
                          in1=erow[:, csl], op0=ALU.is_le, op1=ALU.mult,
                          accum_out=sl_2[:, c:c + 1])

                  # Band col-tiles first: matmul, copy band slice to SBUF,
                  # derive mean_pos, then consume.
                  pheld = []
                  for c in range(nband):
                      p = do_matmul(c)
                      if c == 0:
                          nc.vector.tensor_copy(
                              sband[:, 0:bsplit], p[:, 128 * r:128 * r + bsplit])
                      else:
                          nc.vector.tensor_copy(
                              sband[:, bsplit:w], p[:, 0:w - bsplit])
                      pheld.append(p)
                  bscr = band_pool.tile([128, w], F32, tag="bscr")
                  nc.vector.scalar_tensor_tensor(
                      out=bscr[:], in0=m_band[:], scalar=0.0, in1=sband[:],
                      op0=ALU.add, op1=ALU.mult, accum_out=pos_s[:])
                  nc.vector.tensor_scalar(
                      mp[:], pos_s[:], icnt_r[:, r:r + 1], None, op0=ALU.mult)
                  nc.vector.tensor_scalar(
                      mp2[:], mp[:], SEMI_HARD_MARGIN, None, op0=ALU.subtract)
                  for c in range(nband):
                      consume(c, pheld[c])
                  for c in range(nband, NCT):
                      consume(c, do_matmul(c))

                  # Band corrections (match positions must not count as negs).
                  me = band_pool.tile([128, w], F32, tag="me")
                  g_e = small.tile([128, 1], F32, tag="ge")
                  nc.vector.scalar_tensor_tensor(
                      out=me[:], in0=m_band[:], scalar=0.0, in1=erow[:, bnd],
                      op0=ALU.add, op1=ALU.mult, accum_out=g_e[:])
                  g_1 = small.tile([128, 1], F32, tag="g1")
                  bs1 = band_pool.tile([128, w], F32, tag="bs1")
                  nc.vector.scalar_tensor_tensor(
                      out=bs1[:], in0=sband[:], scalar=mp[:], in1=me[:],
                      op0=ALU.is_lt, op1=ALU.mult, accum_out=g_1[:])
                  g_2 = small.tile([128, 1], F32, tag="g2")
                  bs2 = band_pool.tile([128, w], F32, tag="bs2")
                  nc.vector.scalar_tensor_tensor(
                      out=bs2[:], in0=sband[:], scalar=mp2[:], in1=me[:],
                      op0=ALU.is_le, op1=ALU.mult, accum_out=g_2[:])

                  # neg = sum(sl_e) + sum(sl_1) - sum(sl_2) - g_e - g_1 + g_2
                  red_a = small.tile([128, 1], F32, tag="reda")
                  nc.vector.reduce_sum(out=red_a[:], in_=sl_e[:], axis=AX)
                  red_b = small.tile([128, 1], F32, tag="redb")
                  nc.vector.reduce_sum(out=red_b[:], in_=sl_1[:], axis=AX)
                  red_c = small.tile([128, 1], F32, tag="redc")
                  nc.vector.reduce_sum(out=red_c[:], in_=sl_2[:], axis=AX)
                  t1 = small.tile([128, 1], F32, tag="t1")
                  nc.vector.tensor_tensor(out=t1[:], in0=red_a[:], in1=red_b[:],
                                          op=ALU.add)
                  t2 = small.tile([128, 1], F32, tag="t2")
                  nc.vector.tensor_tensor(out=t2[:], in0=red_c[:], in1=g_e[:],
                                          op=ALU.add)
                  t3 = small.tile([128, 1], F32, tag="t3")
                  nc.vector.tensor_tensor(out=t3[:], in0=t1[:], in1=t2[:],
                                          op=ALU.subtract)
                  t4 = small.tile([128, 1], F32, tag="t4")
                  nc.vector.tensor_tensor(out=t4[:], in0=t3[:], in1=g_1[:],
                                          op=ALU.subtract)
                  neg = small.tile([128, 1], F32, tag="neg")
                  nc.vector.tensor_tensor(out=neg[:], in0=t4[:], in1=g_2[:],
                                          op=ALU.add)

                  # keep terms: sum_match ln(E + neg) - sim/T
                  ea = band_pool.tile([128, w], F32, tag="ea")
                  nc.vector.tensor_scalar(ea[:], erow[:, bnd], neg[:], None,
                                          op0=ALU.add)
                  lg = band_pool.tile([128, w], F32, tag="lg")
                  nc.scalar.activation(lg[:], ea[:], ACTF.Ln)
                  ks_raw = small.tile([128, 1], F32, tag="ksraw")
                  bs3 = band_pool.tile([128, w], F32, tag="bs3")
                  nc.vector.scalar_tensor_tensor(
                      out=bs3[:], in0=m_band[:], scalar=0.0, in1=lg[:],
                      op0=ALU.add, op1=ALU.mult, accum_out=ks_raw[:])
                  pterm = small.tile([128, 1], F32, tag="pterm")
                  nc.vector.tensor_scalar(pterm[:], pos_s[:], invT, None,
                                          op0=ALU.mult)
                  nc.vector.tensor_tensor(
                      out=ks_cols[:, d * NRT + r:d * NRT + r + 1],
                      in0=ks_raw[:], in1=pterm[:], op=ALU.subtract)

        nc.sync.dma_start(ks_out[:], ks_cols[:])

    nc.compile()
    return nc


def _prep(vision_features, text_features, match_ids):
    v = np.ascontiguousarray(np.asarray(vision_features, dtype=np.float32))
    t = np.ascontiguousarray(np.asarray(text_features, dtype=np.float32))
    ids = np.asarray(match_ids).astype(np.int64)

    vn = v / np.maximum(np.linalg.norm(v, axis=1, keepdims=True), EPS)
    tn = t / np.maximum(np.linalg.norm(t, axis=1, keepdims=True), EPS)

    order = np.argsort(ids, kind="stable")
    ids_s = ids[order]
    _, inv, counts = np.unique(ids_s, return_inverse=True, return_counts=True)
    cnt_row = counts[inv].astype(np.int64)  # pos_cnt per sorted row
    m_star = int(cnt_row.max())

    shift = 16
    while m_star > shift + 1:
        shift += 16
    w = 128 + 2 * shift

    vT = np.ascontiguousarray(vn[order].T)  # [D, B]
    tT = np.ascontiguousarray(tn[order].T)
    ids_f = ids_s.astype(np.float32)
    inv_cnt = (1.0 / cnt_row).astype(np.float32)

    in_maps = []
    for core in range(N_CORES):
        roll = shift - core * BLK
        ic = np.roll(ids_f, roll)
        in_maps.append({
            "rhs_t": np.roll(tT, roll, axis=1),
            "rhs_v": np.roll(vT, roll, axis=1),
            "ids_bcd": np.ascontiguousarray(
                np.broadcast_to(ic[:BLK + w], (128, BLK + w))),
            "ids_rows": np.ascontiguousarray(
                ids_f[core * BLK:(core + 1) * BLK].reshape(4, 128).T),
            "inv_cnt": np.ascontiguousarray(
                inv_cnt[core * BLK:(core + 1) * BLK].reshape(4, 128).T),
        })
    meta = {
        "cnt_row": cnt_row,
        "num_pos": int(cnt_row.sum()),
        "valid": (cnt_row > 0) & (cnt_row < B),
        "shift": shift,
        "w": w,
    }
    return in_maps, meta


def _finalize(results, meta):
    ks_v = np.concatenate(
        [r["ks_out"][:, 0:4].T.reshape(-1) for r in results])
    ks_t = np.concatenate(
        [r["ks_out"][:, 4:8].T.reshape(-1) for r in results])
    valid = meta["valid"]
    v2t = np.where(valid, ks_v, 0.0).sum(dtype=np.float64)
    t2v = np.where(valid, ks_t, 0.0).sum(dtype=np.float64)
    num_pos = meta["num_pos"]
    if num_pos > 0:
        loss = (v2t + t2v) / (2.0 * max(num_pos, 1.0))
    else:
        loss = 0.0
    return np.float32(loss)


def kernel(vision_features, text_features, match_ids, _trace=False):
    in_maps, meta = _prep(vision_features, text_features, match_ids)
    key = (meta["shift"], meta["w"])
    if key not in _CACHE:
        _CACHE[key] = _build(*key)
    nc = _CACHE[key]
    res = run_bass_kernel_spmd(nc, in_maps, list(range(N_CORES)),
                               trace=_trace)
    out = _finalize(res.results, meta)
    if _trace:
        return out, res
    return out



# revision 2
# speedup vs baseline: 5.3600x; 5.3600x over previous
"""HardNegativeMiningContrastiveLoss on 8 trn2 NeuronCores (Bass/Tile).

Strategy (v2, fp8):
  - Host: L2-normalize, sort rows by match_id (match matrix becomes a
    narrow diagonal band), scale by 16 and cast to fp8 e4m3, lay out as
    [128, 4 ksub, B] for DoubleRow matmuls. Each core owns a 512-row
    block of anchors for BOTH directions (v2t / t2v); the rhs columns
    are rotated per-core so the match band of local row-tile r sits at
    columns [128r, 128r+w) -- uniform offset, SPMD.
  - Device: sim row-block via fp8 DoubleRow matmuls (contraction 256
    per MM, PSUM fp32, value = 256*sim) into two [128,2048] PSUM slabs;
    one big Exp activation per slab -> erow bf16 in SBUF. Since a
    semi-hard negative (weight 2) is exactly an element counted by both
    thresholds, neg = sum[s<mp]e + sum[s>mp2]e with match entries
    removed via narrow band corrections. Comparisons run in exp space
    (e < exp(mp/T)) so the DVE reads bf16 SBUF at 2x rate. Per
    (direction, row-tile): accum-only stats L, G, -c1, -c2, -g_e,
    pos_s shipped as a [128, 64] tile.
  - Host: ks = cnt*ln(neg) + g_e/neg - mean-pos term (exact to
    O((e/neg)^2) ~ 1e-5), valid-row mask, scalar reduction.
"""

import numpy as np

import concourse.bass as bass
import concourse.bacc as bacc
import concourse.tile as tile
from concourse import mybir
from concourse.bass_utils import run_bass_kernel_spmd
from contextlib import ExitStack

N_CORES = 8
B = 4096
D = 512
BLK = B // N_CORES  # 512 anchors per core
TEMPERATURE = 0.07
SEMI_HARD_MARGIN = 0.2
EPS = 1e-12
FP8_SCALE = 16.0
PSC = FP8_SCALE * FP8_SCALE  # PSUM holds PSC * sim

F32 = mybir.dt.float32
BF16 = mybir.dt.bfloat16
FP8 = mybir.dt.float8e4
ALU = mybir.AluOpType
ACTF = mybir.ActivationFunctionType
PM = mybir.MatmulPerfMode

_CACHE = {}


def _build(shift: int, w: int, repeat: int = 1):
    nc = bacc.Bacc("TRN2", target_bir_lowering=False, debug=False,
                   num_devices=N_CORES)

    rhs_t = nc.dram_tensor("rhs_t", [128, 4, B], FP8, kind="ExternalInput")
    rhs_v = nc.dram_tensor("rhs_v", [128, 4, B], FP8, kind="ExternalInput")
    ids_bcd = nc.dram_tensor("ids_bcd", [128, BLK + w], F32,
                             kind="ExternalInput")
    ids_rows = nc.dram_tensor("ids_rows", [128, 4], F32, kind="ExternalInput")
    icnt_sd = nc.dram_tensor("icnt_s", [128, 4], F32, kind="ExternalInput")
    mrgd = nc.dram_tensor("mrg", [128, 4], F32, kind="ExternalInput")
    stats_out = nc.dram_tensor("stats_out", [128, 64], F32,
                               kind="ExternalOutput")

    invT = 1.0 / TEMPERATURE
    NRT = BLK // 128  # 4 row tiles

    with tile.TileContext(nc) as tc, ExitStack() as ctx:
        rhs_pool = ctx.enter_context(tc.tile_pool(name="rhs", bufs=4))
        e_pool = ctx.enter_context(tc.tile_pool(name="erow", bufs=3))
        psum = ctx.enter_context(
            tc.tile_pool(name="psum", bufs=2, space=bass.MemorySpace.PSUM))
        scr_pool = ctx.enter_context(tc.tile_pool(name="scr", bufs=2))
        band_pool = ctx.enter_context(tc.tile_pool(name="band", bufs=4))
        small = ctx.enter_context(tc.tile_pool(name="small", bufs=4))
        const_pool = ctx.enter_context(tc.tile_pool(name="const", bufs=1))

        ids_bc = const_pool.tile([128, BLK + w], F32, tag="idsbc")
        nc.sync.dma_start(ids_bc[:], ids_bcd[:])
        ids_r = const_pool.tile([128, NRT], F32, tag="idsr")
        nc.sync.dma_start(ids_r[:], ids_rows[:])
        icnt_s = const_pool.tile([128, NRT], F32, tag="icnts")
        nc.sync.dma_start(icnt_s[:], icnt_sd[:])
        mrg = const_pool.tile([128, NRT], F32, tag="mrg")
        nc.sync.dma_start(mrg[:], mrgd[:])

        # Match-mask bands (shared by both directions).
        mb = const_pool.tile([128, NRT * w], BF16, tag="mb")
        for r in range(NRT):
            nc.vector.tensor_scalar(
                mb[:, r * w:(r + 1) * w], ids_bc[:, 128 * r:128 * r + w],
                ids_r[:, r:r + 1], None, op0=ALU.is_equal)

        out_t = const_pool.tile([128, 64], F32, tag="outt")

        for rep in range(repeat):
            rv = rhs_pool.tile([128, 4, B], FP8, tag="rv")
            nc.sync.dma_start(rv[:], rhs_v[:])
            rt = rhs_pool.tile([128, 4, B], FP8, tag="rt")
            nc.sync.dma_start(rt[:], rhs_t[:])

            for d in range(2):
                mov = rt if d == 0 else rv   # moving: all columns
                sta = rv if d == 0 else rt   # stationary: anchor slices

                for r in range(NRT):
                    u = 4 * d + r
                    oc = 8 * u
                    erow = e_pool.tile([128, B], BF16, tag="erow")
                    em = small.tile([128, 2], F32, tag="em")

                    for half in range(2):
                        p_big = psum.tile([128, 2048], F32, tag="p")
                        for c in range(4):
                            cs = 2048 * half + 512 * c
                            for j in range(2):
                                nc.tensor.matmul(
                                    p_big[:, 512 * c:512 * c + 512],
                                    sta[:, 2 * j:2 * j + 2,
                                        shift + 128 * r:shift + 128 * r + 128],
                                    mov[:, 2 * j:2 * j + 2, cs:cs + 512],
                                    start=(j == 0), stop=(j == 1),
                                    perf_mode=PM.DoubleRow)
                        nc.scalar.activation(
                            erow[:, 2048 * half:2048 * (half + 1)], p_big[:],
                            ACTF.Exp, scale=invT / PSC)
                        if half == 0:
                            # pos_s = sum_match P over the band (sim space).
                            bscr = band_pool.tile([128, w], F32, tag="bscr")
                            nc.vector.scalar_tensor_tensor(
                                out=bscr[:], in0=mb[:, r * w:(r + 1) * w],
                                scalar=1.0,
                                in1=p_big[:, 128 * r:128 * r + w],
                                op0=ALU.mult, op1=ALU.mult,
                                accum_out=out_t[:, oc:oc + 1])
                            nc.vector.tensor_scalar(
                                out_t[:, oc + 1:oc + 2], out_t[:, oc:oc + 1],
                                mrg[:, r:r + 1], None, op0=ALU.subtract)
                            # [emp, emp2] = Exp((pos_s, pos_s-mrg) * icnt_s)
                            nc.scalar.activation(
                                em[:], out_t[:, oc:oc + 2], ACTF.Exp,
                                scale=icnt_s[:, r:r + 1])

                    # Band corrections (match entries out of L+G).
                    eb = erow[:, 128 * r:128 * r + w]
                    me = band_pool.tile([128, w], BF16, tag="me")
                    nc.vector.scalar_tensor_tensor(
                        out=me[:], in0=mb[:, r * w:(r + 1) * w], scalar=-1.0,
                        in1=eb, op0=ALU.mult, op1=ALU.mult,
                        accum_out=out_t[:, oc + 6:oc + 7])
                    bs1 = band_pool.tile([128, w], BF16, tag="bs1")
                    nc.vector.scalar_tensor_tensor(
                        out=bs1[:], in0=eb, scalar=em[:, 0:1], in1=me[:],
                        op0=ALU.is_lt, op1=ALU.mult,
                        accum_out=out_t[:, oc + 4:oc + 5])
                    bs2 = band_pool.tile([128, w], BF16, tag="bs2")
                    nc.vector.scalar_tensor_tensor(
                        out=bs2[:], in0=eb, scalar=em[:, 1:2], in1=me[:],
                        op0=ALU.is_gt, op1=ALU.mult,
                        accum_out=out_t[:, oc + 5:oc + 6])
                    # Full-row masked sums: L = sum[e<emp]e, G = sum[e>emp2]e.
                    s1 = scr_pool.tile([128, B], BF16, tag="s1")
                    nc.vector.scalar_tensor_tensor(
                        out=s1[:], in0=erow[:], scalar=em[:, 0:1],
                        in1=erow[:], op0=ALU.is_lt, op1=ALU.mult,
                        accum_out=out_t[:, oc + 2:oc + 3])
                    s2 = scr_pool.tile([128, B], BF16, tag="s2")
                    nc.vector.scalar_tensor_tensor(
                        out=s2[:], in0=erow[:], scalar=em[:, 1:2],
                        in1=erow[:], op0=ALU.is_gt, op1=ALU.mult,
                        accum_out=out_t[:, oc + 3:oc + 4])

        nc.sync.dma_start(stats_out[:], out_t[:])

    nc.compile()
    return nc


def _prep(vision_features, text_features, match_ids):
    v = np.ascontiguousarray(np.asarray(vision_features, dtype=np.float32))
    t = np.ascontiguousarray(np.asarray(text_features, dtype=np.float32))
    ids = np.asarray(match_ids).astype(np.int64)

    vn = v / np.maximum(np.linalg.norm(v, axis=1, keepdims=True), EPS)
    tn = t / np.maximum(np.linalg.norm(t, axis=1, keepdims=True), EPS)

    order = np.argsort(ids, kind="stable")
    ids_s = ids[order]
    _, inv, counts = np.unique(ids_s, return_inverse=True, return_counts=True)
    cnt_row = counts[inv].astype(np.int64)
    m_star = int(cnt_row.max())

    shift = 16
    while m_star > shift + 1:
        shift += 16
    w = 128 + 2 * shift

    f8 = mybir.dt.np(FP8)
    vq = (vn[order].T * FP8_SCALE).astype(f8)  # [D, B]
    tq = (tn[order].T * FP8_SCALE).astype(f8)
    ids_f = ids_s.astype(np.float32)
    cnt_f = cnt_row.astype(np.float32)

    in_maps = []
    for core in range(N_CORES):
        roll = shift - core * BLK
        ic = np.roll(ids_f, roll)

        def lay(a):
            ar = np.roll(a, roll, axis=1)  # [D, B]
            return np.ascontiguousarray(
                ar.reshape(4, 128, B).transpose(1, 0, 2))

        blk = slice(core * BLK, (core + 1) * BLK)
        in_maps.append({
            "rhs_t": lay(tq),
            "rhs_v": lay(vq),
            "ids_bcd": np.ascontiguousarray(
                np.broadcast_to(ic[:BLK + w], (128, BLK + w))),
            "ids_rows": np.ascontiguousarray(
                ids_f[blk].reshape(4, 128).T),
            "icnt_s": np.ascontiguousarray(
                (1.0 / (TEMPERATURE * PSC * cnt_f[blk])).reshape(4, 128).T),
            "mrg": np.ascontiguousarray(
                (PSC * SEMI_HARD_MARGIN * cnt_f[blk]).reshape(4, 128).T),
        })
    meta = {
        "cnt_row": cnt_row,
        "num_pos": int(cnt_row.sum()),
        "valid": (cnt_row > 0) & (cnt_row < B),
        "shift": shift,
        "w": w,
    }
    return in_maps, meta


def _finalize(results, meta):
    cnt = meta["cnt_row"].astype(np.float64)
    valid = meta["valid"]
    invT = 1.0 / TEMPERATURE
    tot = 0.0
    for d in range(2):
        for core, res in enumerate(results):
            st = res["stats_out"].astype(np.float64)  # [128, 64]
            for r in range(4):
                oc = 8 * (4 * d + r)
                pos_s = st[:, oc]
                L, G = st[:, oc + 2], st[:, oc + 3]
                c1n, c2n, g_en = st[:, oc + 4], st[:, oc + 5], st[:, oc + 6]
                neg = L + G + c1n + c2n
                g_e = -g_en
                rows = slice(core * BLK + r * 128, core * BLK + r * 128 + 128)
                c = cnt[rows]
                ks = c * np.log(np.maximum(neg, 1e-300)) + g_e / np.maximum(
                    neg, 1e-300) - pos_s * (invT / PSC)
                tot += np.where(valid[rows], ks, 0.0).sum()
    num_pos = meta["num_pos"]
    if num_pos > 0:
        loss = tot / (2.0 * max(num_pos, 1.0))
    else:
        loss = 0.0
    return np.float32(loss)


def kernel(vision_features, text_features, match_ids, _trace=False):
    in_maps, meta = _prep(vision_features, text_features, match_ids)
    key = (meta["shift"], meta["w"])
    if key not in _CACHE:
        _CACHE[key] = _build(*key)
    nc = _CACHE[key]
    res = run_bass_kernel_spmd(nc, in_maps, list(range(N_CORES)),
                               trace=_trace)
    out = _finalize(res.results, meta)
    if _trace:
        return out, res
    return out
